# revision 21
# baseline (speedup 1.0000x reference)
"""DiffLBP soft-histogram kernel for Trainium2 (8 NeuronCores).

Math: the per-pixel softmax over 256 LBP patterns factorizes exactly into a
product of 8 independent Bernoullis with q_p = 1/2 (1 + z_p),
z_p = tanh((oh/2)*tanh(dh*d_p)).  The histogram is a 16x16 Gram matrix of
z-monomials (4 low bits x 4 high bits) pushed through a constant Walsh +-1
transform (host).  Antipodal offsets give z_{p+4}(r,c) = -z_p((r,c)+off), so
only 4 z planes are computed; the "hi" side needs (row+1, col+dx) shifted
copies: both shifts are done by TensorE (subdiagonal-identity matmul with
col-offset APs into PSUM), drained to SBUF by one ScalarE copy per tile.

Device program per core (one batch b, one 255-row half; tile 1 first):
  z phase (x2 tiles of 128 rows): SWDGE DMA loads XA/XB row-windows cast to
    bf16; DVE computes the 4 diffs; ScalarE does two tanh passes, writing
    the z singles straight into the plane-major mlo tile (planes 1..4).
  shift (x2): TensorE multiplies z planes 0..2 by a subdiagonal identity
    with per-plane column offsets -> PSUM holds the row+col shifted hi
    singles (tile 0 also accumulates a selector matmul that injects
    z-tile-1 row 0 into row 127); ScalarE drains PSUM -> mhi planes 1..3.
  mono phase (x2): both mlo and mhi are plane-major [128, 16, 512]; the 11
    composite monomial planes are built by 4 batched DVE multiplies + 1
    GPSIMD multiply ({03}, which needs only singles so it runs early).
  gram phase (x2): 64 matmuls accumulate into PSUM (strided lhsT selects
    16 planes x 8 cols).  A zero-weight matmul warmup stream keeps the PE
    HAM un-throttled before the gram bursts.  Tile 0 runs K=128 (the
    straddle row center 128 is included on-device), tile 1 K=127 (its row
    127 is the neighbouring core's center).  No host boundary fix.
"""

import os
import numpy as np
from contextlib import ExitStack

H = W = 512
HP = WP = 510          # valid center rows/cols
NROWS_SLICE = 257      # input rows per core slice

# plane slot -> subset bitmask of {z0,z1,z2,z3}; chosen so the 11 composite
# planes are produced by 5 batched multiplies (see _products)
PLANE_SUBSET = [0b0000,
                0b0001, 0b0010, 0b0100, 0b1000,   # 1..4:   z0 z1 z2 z3
                0b0011, 0b0110, 0b1100,           # 5..7:   {01} {12} {23}
                0b0101, 0b1010,                   # 8..9:   {02} {13}
                0b0111, 0b1110, 0b1111,           # 10..12: {012} {123} {0123}
                0b1011, 0b1101,                   # 13..14: {013} {023}
                0b1001]                           # 15:     {03}

_PROGRAM_CACHE = {}
last_results = None  # BassKernelResults of the most recent run (for test harness)


def _products(nc, m, R):
    """Emit the 11 composite monomial planes from singles (planes 1..4) of a
    plane-major view m[[part], 16, W]; writes planes 5..15 on R partitions.
    """
    nc.vector.tensor_mul(m[:R, 15:16], m[:R, 1:2], m[:R, 4:5])      # 03
    nc.vector.tensor_mul(m[:R, 5:8], m[:R, 1:4], m[:R, 2:5])        # 01 12 23
    nc.vector.tensor_mul(m[:R, 8:10], m[:R, 1:3], m[:R, 3:5])       # 02 13
    nc.vector.tensor_mul(m[:R, 10:13], m[:R, 5:8], m[:R, 3:6])      # 012 123 0123
    nc.vector.tensor_mul(m[:R, 13:15], m[:R, 9:7:-1], m[:R, 1:5:3]) # 013 023


def _build_program(dh: float, oh: float):
    import concourse.bacc as bacc
    import concourse.tile as tile
    from concourse import mybir
    import concourse.bass as bass

    f32 = mybir.dt.float32
    bf16 = mybir.dt.bfloat16
    Tanh = mybir.ActivationFunctionType.Tanh
    Copy = mybir.ActivationFunctionType.Copy

    nc = bacc.Bacc("TRN2", target_bir_lowering=False, debug=False)
    xs_t = nc.dram_tensor("xs", [NROWS_SLICE, W], f32, kind="ExternalInput")
    id_t = nc.dram_tensor("ident", [128, 256], bf16, kind="ExternalInput")
    gram = nc.dram_tensor("gram", [128, 128], f32, kind="ExternalOutput").ap()

    with tile.TileContext(nc) as tc, ExitStack() as ctx:
        xpool = ctx.enter_context(tc.tile_pool(name="x", bufs=2))
        dpool = ctx.enter_context(tc.tile_pool(name="d", bufs=2))
        tpool = ctx.enter_context(tc.tile_pool(name="t", bufs=1))
        mpool = ctx.enter_context(tc.tile_pool(name="m", bufs=1))
        ppool = ctx.enter_context(
            tc.tile_pool(name="ps", bufs=1, space=bass.MemorySpace.PSUM))

        # x loads first (they gate everything); SWDGE casts f32 -> bf16
        xts = {}
        for i in (1, 0):
            xt = xpool.tile([128, 2, W], bf16, name=f"xt{i}", tag=f"xt{i}")
            src = bass.AP(xs_t, 128 * i * W, [[W, 128], [W, 2], [1, W]])
            nc.gpsimd.dma_start(xt[:], src)
            xts[i] = xt

        # shifted-identity weights for the TensorE partition shift
        identt = mpool.tile([128, 256], bf16, tag="identt")
        nc.sync.dma_start(identt[:, :], id_t.ap())

        # trigger the tanh ACT table load immediately (overlaps the X DMAs)
        warm = mpool.tile([1, 8], f32, tag="warm")
        nc.vector.memset(warm[:, :], 0.0)
        nc.scalar.activation(warm[:, :], warm[:, :], Tanh)

        # zero stationary for the PE HAM warmup (contributes 0 to the gram)
        wz = mpool.tile([128, 128], bf16, tag="wz")
        nc.vector.memset(wz[:, :], 0.0)

        insts = {}
        ps = ppool.tile([128, 128], f32, tag="ps")
        pshift = {i: ppool.tile([128, 3, W], f32, name=f"pshift{i}",
                                tag=f"pshift{i}") for i in (0, 1)}
        mlos, mhis = {}, {}
        KS = {0: 128, 1: 127}   # gram contraction depth per tile

        # mlo: plane-major (the gram's MOVING operand tolerates 2 free
        # dims); z singles live in planes 1..4.  mhi: chunk-major [128, 64
        # chunks, 16 planes x 8 cols] (the STATIONARY needs 1 free dim).
        mhv = {}
        for i in (1, 0):
            mlos[i] = mpool.tile([128, 16, W], bf16, name=f"mlo{i}", tag=f"mlo{i}")
            mhis[i] = mpool.tile([128, 64, 128], bf16, name=f"mhi{i}", tag=f"mhi{i}")
            mhv[i] = mhis[i][:].rearrange("k g (s c) -> k s g c", c=8)
            nc.gpsimd.memset(mlos[i][:, 0, :], 1.0)
            nc.gpsimd.memset(mhis[i][:, :, 0:8], 1.0)

        def z_phase(i):
            xt = xts[i]
            xa, xb = xt[:, 0, :], xt[:, 1, :]
            d = dpool.tile([128, 4, W], bf16, name=f"d{i}", tag=f"d{i}")
            # zero the columns the subs below don't write (else NaN garbage
            # flows through tanh into the z edge cols)
            nc.vector.memset(d[:, 0, 0:1], 0.0)
            nc.vector.memset(d[:, 2:4, 511:512], 0.0)
            # d_p[cl] = X_{dy}[cl+dx] - XB[cl]   (cl = x-col = center_col + 1)
            nc.vector.tensor_sub(d[:, 0, 1:512], xa[:, 0:511], xb[:, 1:512])   # (-1,-1)
            nc.vector.tensor_sub(d[:, 1, 0:512], xa[:, 0:512], xb[:, 0:512])   # (-1, 0)
            nc.vector.tensor_sub(d[:, 2, 0:511], xa[:, 1:512], xb[:, 0:511])   # (-1,+1)
            nc.vector.tensor_sub(d[:, 3, 0:511], xb[:, 1:512], xb[:, 0:511])   # ( 0,+1)

            t = tpool.tile([128, 4, W], f32, name=f"t{i}", tag="t")
            insts[f"t{i}"] = nc.scalar.activation(
                t[:, :, :], d[:, :, :], Tanh, scale=float(dh))
            # z singles straight into the plane-major mlo tile
            nc.scalar.activation(mlos[i][:, 1:5, :], t[:, :, :], Tanh,
                                 scale=float(oh) / 2.0)

        # col windows for the shift matmuls: plane p shifts cols by
        # dx'_p = (+1, 0, -1); out col c reads z col c+dx'
        SHIFT_WIN = [  # (out_lo, out_hi, in_lo, in_hi)
            (0, 511, 1, 512),
            (0, 512, 0, 512),
            (1, 512, 0, 511),
        ]

        def shift_phase(i):
            # pshift[i][rl, p, c] = z_p(row rl+1, col c+dx') via TensorE;
            # tile 0 row 127 = z-tile-1 row 0 via the selector weights.
            for p, (ol, ohi, il, ihi) in enumerate(SHIFT_WIN):
                nc.tensor.matmul(pshift[i][:, p, ol:ohi], identt[:, 0:128],
                                 mlos[i][:, 1 + p, il:ihi],
                                 start=True, stop=(i == 1))
            if i == 0:
                for p, (ol, ohi, il, ihi) in enumerate(SHIFT_WIN):
                    nc.tensor.matmul(pshift[0][:, p, ol:ohi], identt[:, 128:256],
                                     mlos[1][:, 1 + p, il:ihi],
                                     start=False, stop=True)

        def mono_phase(i):
            mlo, mh = mlos[i], mhv[i]
            R = KS[i]
            _products(nc, mlo[:], 128)
            # plane 4 of the hi side = z_3 col-shifted by -1, copied from mlo
            # plane 4 BEFORE the edge memset (the hi side needs the real
            # z3(cl=0); the copy's src col -1 lands on the cl=0 slot, which
            # the mhi edge memset overwrites)
            psr = pshift[i][0:R, :, :].rearrange("k p (g c) -> k p g c", c=8)
            p4src = (mlo[:, 3:5, :].rearrange("k p c -> k (p c)")[:, 511:1023]
                     .rearrange("k (g c) -> k g c", c=8))
            nc.vector.tensor_copy(mh[:, 4, :, :], p4src)
            # invalid edge columns: zero ALL lo planes at cl=0 and cl=511
            # (after products; kills the unwritten-d edge garbage too)
            nc.vector.memset(mlo[:, :, 0:512:511], 0.0)
            # hi singles 1..3: drain the shift PSUM (already row+col shifted)
            insts[f"drain{i}"] = nc.scalar.activation(mh[0:R, 1:4, :, :], psr, Copy)
            nc.vector.memset(mhis[i][:, 0, 8:40:8], 0.0)    # cl=0   singles
            nc.vector.memset(mhis[i][:, 63, 15:47:8], 0.0)  # cl=511 singles
            if i == 1:
                _products(nc, mh, R)
            else:
                # split tile 0's hi products by chunk half so the gram tail
                # can start after half A
                _products(nc, mh[:, :, 0:32, :], R)
                _products(nc, mh[:, :, 32:64, :], R)

        def warmup_phase(n, src_i, first=False):
            # zero-weight matmuls into the gram PSUM: keeps the PE HAM busy
            # so the gram bursts run at 2.4 GHz; contributes exactly 0.  The
            # moving operand reads z data so the warmup can't run too early.
            for k in range(n):
                nc.tensor.matmul(ps[:, :], wz[:, :], mlos[src_i][:, 2, 1:129],
                                 start=(first and k == 0), stop=False)

        def gram_phase(i, last, gs=range(64)):
            K = KS[i]
            for g in gs:
                nc.tensor.matmul(
                    ps[:, :],
                    mhis[i][0:K, g, :],
                    mlos[i][0:K, :, 8 * g:8 * g + 8],
                    start=False,
                    stop=(last and g == 63),
                )

        # tile 1 first: its shift is self-contained, so its gram unblocks
        # early; tile 0's shift needs z-tile-1 row 0 (the selector matmul)
        z_phase(1)
        z_phase(0)
        shift_phase(1)
        warmup_phase(50, 1, first=True)
        shift_phase(0)
        with tc.high_priority():
            mono_phase(1)
            gram_phase(1, False)
        from concourse.tile import add_dep_helper
        add_dep_helper(insts["t0"].ins, insts["drain1"].ins, sync=True,
                       reason="slot the tile-1 PSUM drain before t0 on ACT")
        mono_phase(0)
        gram_phase(0, False, range(0, 32))
        gram_phase(0, True, range(32, 64))

        gout = mpool.tile([128, 128], f32, tag="gout")
        nc.vector.tensor_copy(gout[:, :], ps[:, :])
        nc.sync.dma_start(gram[:, :], gout[:, :])

    nc.compile()
    return nc


def _walsh16():
    sgn = 2.0 * ((np.arange(16)[:, None] >> np.arange(4)[None, :]) & 1) - 1.0
    w = np.ones((16, 16))
    for k in range(16):
        for s in range(16):
            v = 1.0
            for p in range(4):
                if s >> p & 1:
                    v *= sgn[k, p]
            w[k, s] = v
    return w


def _postprocess(grams):
    """grams: 8x [128,128] f32 -> [4,256,1,1]."""
    perm = np.argsort(PLANE_SUBSET)  # subset-index -> plane-index
    w16 = _walsh16()
    hi_sign = np.array([(-1.0) ** bin(s).count("1") for s in range(16)])
    out = np.zeros((4, 256), np.float64)
    for b in range(4):
        g16 = np.zeros((16, 16))
        for half in range(2):
            gr = grams[2 * b + half].astype(np.float64).reshape(16, 8, 16, 8)
            g = np.einsum("tgsg->st", gr)          # sum the 8 diagonal blocks
            g16 += g[np.ix_(perm, perm)]           # plane order -> subset order
        g16 *= hi_sign[None, :]                    # SA planes hold -z_{p+4}
        g16[0, 0] = float(HP * WP)                 # ones*ones: exact pixel count
        hmat = 2.0 ** -8 * (w16 @ g16 @ w16.T)     # [klo, khi]
        out[b] = hmat.T.reshape(256)               # k = klo + 16*khi
    return out.astype(np.float32).reshape(4, 256, 1, 1)


def _ident_np():
    import ml_dtypes
    a = np.zeros((128, 256), dtype=np.float32)
    for m in range(127):
        a[m + 1, m] = 1.0          # subdiagonal: out[m] = z[m+1]
    a[0, 128 + 127] = 1.0          # selector: out[127] = other-tile z[0]
    return a.astype(ml_dtypes.bfloat16)


def kernel(x, diff_hardness, output_hardness):
    global last_results
    from concourse.bass_utils import run_bass_kernel_spmd

    x = np.asarray(x, np.float32)
    dh = float(np.asarray(diff_hardness))
    oh = float(np.asarray(output_hardness))

    key = (dh, oh)
    if key not in _PROGRAM_CACHE:
        _PROGRAM_CACHE[key] = _build_program(dh, oh)
    nc = _PROGRAM_CACHE[key]

    ident = _ident_np()
    in_maps = []
    for core in range(8):
        b, half = divmod(core, 2)
        r0 = 0 if half == 0 else 255
        in_maps.append({
            "xs": np.ascontiguousarray(x[b, 0, r0:r0 + NROWS_SLICE, :]),
            "ident": ident,
        })

    trace = bool(int(os.environ.get("KERNEL_TRACE", "0")))
    res = run_bass_kernel_spmd(nc, in_maps, core_ids=list(range(8)), trace=trace)
    last_results = res
    grams = [res.results[c]["gram"] for c in range(8)]
    return _postprocess(grams)


# revision 22
# speedup vs baseline: 1.0025x; 1.0025x over previous
"""DiffLBP soft-histogram kernel for Trainium2 (8 NeuronCores).

Math: the per-pixel softmax over 256 LBP patterns factorizes exactly into a
product of 8 independent Bernoullis with q_p = 1/2 (1 + z_p),
z_p = tanh((oh/2)*tanh(dh*d_p)).  The histogram is a 16x16 Gram matrix of
z-monomials (4 low bits x 4 high bits) pushed through a constant Walsh +-1
transform (host).  Antipodal offsets give z_{p+4}(r,c) = -z_p((r,c)+off), so
only 4 z planes are computed; the "hi" side needs (row+1, col+dx) shifted
copies: both shifts are done by TensorE (subdiagonal-identity matmul with
col-offset APs into PSUM), drained to SBUF by one ScalarE copy per tile.

Device program per core (one batch b, one 255-row half; tile 1 first):
  z phase (x2 tiles of 128 rows): SWDGE DMA loads XA/XB row-windows cast to
    bf16; DVE computes the 4 diffs; ScalarE does two tanh passes, writing
    the z singles straight into the plane-major mlo tile (planes 1..4).
  shift (x2): TensorE multiplies z planes 0..2 by a subdiagonal identity
    with per-plane column offsets -> PSUM holds the row+col shifted hi
    singles (tile 0 also accumulates a selector matmul that injects
    z-tile-1 row 0 into row 127); ScalarE drains PSUM -> mhi planes 1..3.
  mono phase (x2): mlo is plane-major [128, 16, 512] (the gram's moving
    operand), mhi chunk-major [128, 64, 16x8] (the stationary needs a
    single-free-dim AP); the 11 composite monomial planes per side are
    built by 5 batched DVE multiplies.  The tile-1 PSUM drain is pinned
    between z1 and t0 on ScalarE via an explicit dep so the hi-side chain
    never waits behind the tile-0 tanh passes.
  gram phase (x2): 64 matmuls per tile accumulate into PSUM (tile-0's hi
    products and gram are split into chunk halves to shorten the tail).
    A zero-weight matmul warmup stream (reading z data so it cannot run
    too early) keeps the PE HAM un-throttled through the gram bursts.
    Tile 0 runs K=128 (the straddle row center 128 is included on-device:
    a selector matmul injects z-tile-1 row 0 into shift row 127), tile 1
    K=127 (its row 127 is the neighbouring core's center).  No host
    boundary fix needed.
"""

import os
import numpy as np
from contextlib import ExitStack

H = W = 512
HP = WP = 510          # valid center rows/cols
NROWS_SLICE = 257      # input rows per core slice

# plane slot -> subset bitmask of {z0,z1,z2,z3}; chosen so the 11 composite
# planes are produced by 5 batched multiplies (see _products)
PLANE_SUBSET = [0b0000,
                0b0001, 0b0010, 0b0100, 0b1000,   # 1..4:   z0 z1 z2 z3
                0b0011, 0b0110, 0b1100,           # 5..7:   {01} {12} {23}
                0b0101, 0b1010,                   # 8..9:   {02} {13}
                0b0111, 0b1110, 0b1111,           # 10..12: {012} {123} {0123}
                0b1011, 0b1101,                   # 13..14: {013} {023}
                0b1001]                           # 15:     {03}

_PROGRAM_CACHE = {}
last_results = None  # BassKernelResults of the most recent run (for test harness)


def _products(nc, m, R):
    """Emit the 11 composite monomial planes from singles (planes 1..4) of a
    plane-major view m[[part], 16, W]; writes planes 5..15 on R partitions.
    """
    nc.vector.tensor_mul(m[:R, 15:16], m[:R, 1:2], m[:R, 4:5])      # 03
    nc.vector.tensor_mul(m[:R, 5:8], m[:R, 1:4], m[:R, 2:5])        # 01 12 23
    nc.vector.tensor_mul(m[:R, 8:10], m[:R, 1:3], m[:R, 3:5])       # 02 13
    nc.vector.tensor_mul(m[:R, 10:13], m[:R, 5:8], m[:R, 3:6])      # 012 123 0123
    nc.vector.tensor_mul(m[:R, 13:15], m[:R, 9:7:-1], m[:R, 1:5:3]) # 013 023


def _build_program(dh: float, oh: float):
    import concourse.bacc as bacc
    import concourse.tile as tile
    from concourse import mybir
    import concourse.bass as bass

    f32 = mybir.dt.float32
    bf16 = mybir.dt.bfloat16
    Tanh = mybir.ActivationFunctionType.Tanh
    Copy = mybir.ActivationFunctionType.Copy

    nc = bacc.Bacc("TRN2", target_bir_lowering=False, debug=False)
    xs_t = nc.dram_tensor("xs", [NROWS_SLICE, W], f32, kind="ExternalInput")
    id_t = nc.dram_tensor("ident", [128, 256], bf16, kind="ExternalInput")
    gram = nc.dram_tensor("gram", [128, 128], f32, kind="ExternalOutput").ap()

    with tile.TileContext(nc) as tc, ExitStack() as ctx:
        xpool = ctx.enter_context(tc.tile_pool(name="x", bufs=2))
        dpool = ctx.enter_context(tc.tile_pool(name="d", bufs=2))
        tpool = ctx.enter_context(tc.tile_pool(name="t", bufs=1))
        mpool = ctx.enter_context(tc.tile_pool(name="m", bufs=1))
        ppool = ctx.enter_context(
            tc.tile_pool(name="ps", bufs=1, space=bass.MemorySpace.PSUM))

        # x loads first (they gate everything); SWDGE casts f32 -> bf16
        xts = {}
        for i in (1, 0):
            xt = xpool.tile([128, 2, W], bf16, name=f"xt{i}", tag=f"xt{i}")
            src = bass.AP(xs_t, 128 * i * W, [[W, 128], [W, 2], [1, W]])
            nc.gpsimd.dma_start(xt[:], src)
            xts[i] = xt

        # shifted-identity weights for the TensorE partition shift
        identt = mpool.tile([128, 256], bf16, tag="identt")
        nc.sync.dma_start(identt[:, :], id_t.ap())

        # trigger the tanh ACT table load immediately (overlaps the X DMAs)
        warm = mpool.tile([1, 8], f32, tag="warm")
        nc.vector.memset(warm[:, :], 0.0)
        nc.scalar.activation(warm[:, :], warm[:, :], Tanh)

        # zero stationary for the PE HAM warmup (contributes 0 to the gram)
        wz = mpool.tile([128, 128], bf16, tag="wz")
        nc.vector.memset(wz[:, :], 0.0)

        insts = {}
        ps = ppool.tile([128, 128], f32, tag="ps")
        pshift = {i: ppool.tile([128, 3, W], f32, name=f"pshift{i}",
                                tag=f"pshift{i}") for i in (0, 1)}
        mlos, mhis = {}, {}
        KS = {0: 128, 1: 127}   # gram contraction depth per tile

        # mlo: plane-major (the gram's MOVING operand tolerates 2 free
        # dims); z singles live in planes 1..4.  mhi: chunk-major [128, 64
        # chunks, 16 planes x 8 cols] (the STATIONARY needs 1 free dim).
        mhv = {}
        for i in (1, 0):
            mlos[i] = mpool.tile([128, 16, W], bf16, name=f"mlo{i}", tag=f"mlo{i}")
            mhis[i] = mpool.tile([128, 64, 128], bf16, name=f"mhi{i}", tag=f"mhi{i}")
            mhv[i] = mhis[i][:].rearrange("k g (s c) -> k s g c", c=8)
            nc.gpsimd.memset(mlos[i][:, 0, :], 1.0)
            nc.gpsimd.memset(mhis[i][:, :, 0:8], 1.0)

        def z_phase(i):
            xt = xts[i]
            xa, xb = xt[:, 0, :], xt[:, 1, :]
            d = dpool.tile([128, 4, W], bf16, name=f"d{i}", tag=f"d{i}")
            # zero the columns the subs below don't write (else NaN garbage
            # flows through tanh into the z edge cols)
            nc.vector.memset(d[:, 0, 0:1], 0.0)
            nc.vector.memset(d[:, 2:4, 511:512], 0.0)
            # d_p[cl] = X_{dy}[cl+dx] - XB[cl]   (cl = x-col = center_col + 1)
            nc.vector.tensor_sub(d[:, 0, 1:512], xa[:, 0:511], xb[:, 1:512])   # (-1,-1)
            nc.vector.tensor_sub(d[:, 1, 0:512], xa[:, 0:512], xb[:, 0:512])   # (-1, 0)
            nc.vector.tensor_sub(d[:, 2, 0:511], xa[:, 1:512], xb[:, 0:511])   # (-1,+1)
            nc.vector.tensor_sub(d[:, 3, 0:511], xb[:, 1:512], xb[:, 0:511])   # ( 0,+1)

            t = tpool.tile([128, 4, W], f32, name=f"t{i}", tag="t")
            insts[f"t{i}"] = nc.scalar.activation(
                t[:, :, :], d[:, :, :], Tanh, scale=float(dh))
            # z singles straight into the plane-major mlo tile
            nc.scalar.activation(mlos[i][:, 1:5, :], t[:, :, :], Tanh,
                                 scale=float(oh) / 2.0)

        # col windows for the shift matmuls: plane p shifts cols by
        # dx'_p = (+1, 0, -1); out col c reads z col c+dx'
        SHIFT_WIN = [  # (out_lo, out_hi, in_lo, in_hi)
            (0, 511, 1, 512),
            (0, 512, 0, 512),
            (1, 512, 0, 511),
        ]

        def shift_phase(i):
            # pshift[i][rl, p, c] = z_p(row rl+1, col c+dx') via TensorE;
            # tile 0 row 127 = z-tile-1 row 0 via the selector weights.
            for p, (ol, ohi, il, ihi) in enumerate(SHIFT_WIN):
                nc.tensor.matmul(pshift[i][:, p, ol:ohi], identt[:, 0:128],
                                 mlos[i][:, 1 + p, il:ihi],
                                 start=True, stop=(i == 1))
            if i == 0:
                for p, (ol, ohi, il, ihi) in enumerate(SHIFT_WIN):
                    nc.tensor.matmul(pshift[0][:, p, ol:ohi], identt[:, 128:256],
                                     mlos[1][:, 1 + p, il:ihi],
                                     start=False, stop=True)

        def mono_phase(i):
            mlo, mh = mlos[i], mhv[i]
            R = KS[i]
            _products(nc, mlo[:], 128)
            # plane 4 of the hi side = z_3 col-shifted by -1, copied from mlo
            # plane 4 BEFORE the edge memset (the hi side needs the real
            # z3(cl=0); the copy's src col -1 lands on the cl=0 slot, which
            # the mhi edge memset overwrites)
            psr = pshift[i][0:R, :, :].rearrange("k p (g c) -> k p g c", c=8)
            p4src = (mlo[:, 3:5, :].rearrange("k p c -> k (p c)")[:, 511:1023]
                     .rearrange("k (g c) -> k g c", c=8))
            nc.vector.tensor_copy(mh[:, 4, :, :], p4src)
            # invalid edge columns: zero ALL lo planes at cl=0 and cl=511
            # (after products; kills the unwritten-d edge garbage too)
            nc.vector.memset(mlo[:, :, 0:512:511], 0.0)
            # hi singles 1..3: drain the shift PSUM (already row+col shifted)
            insts[f"drain{i}"] = nc.scalar.activation(mh[0:R, 1:4, :, :], psr, Copy)
            nc.vector.memset(mhis[i][:, 0, 8:40:8], 0.0)    # cl=0   singles
            nc.vector.memset(mhis[i][:, 63, 15:47:8], 0.0)  # cl=511 singles
            if i == 1:
                _products(nc, mh, R)
            else:
                # split tile 0's hi products by chunk half so the gram tail
                # can start after half A
                _products(nc, mh[:, :, 0:32, :], R)
                _products(nc, mh[:, :, 32:64, :], R)

        def warmup_phase(n, src_i, first=False):
            # zero-weight matmuls into the gram PSUM: keeps the PE HAM busy
            # so the gram bursts run at 2.4 GHz; contributes exactly 0.  The
            # moving operand reads z data so the warmup can't run too early.
            for k in range(n):
                nc.tensor.matmul(ps[:, :], wz[:, :], mlos[src_i][:, 2, 1:129],
                                 start=(first and k == 0), stop=False)

        def gram_phase(i, last, gs=range(64)):
            K = KS[i]
            for g in gs:
                nc.tensor.matmul(
                    ps[:, :],
                    mhis[i][0:K, g, :],
                    mlos[i][0:K, :, 8 * g:8 * g + 8],
                    start=False,
                    stop=(last and g == 63),
                )

        # tile 1 first: its shift is self-contained, so its gram unblocks
        # early; tile 0's shift needs z-tile-1 row 0 (the selector matmul)
        z_phase(1)
        z_phase(0)
        shift_phase(1)
        warmup_phase(50, 1, first=True)
        shift_phase(0)
        with tc.high_priority():
            mono_phase(1)
            gram_phase(1, False)
        from concourse.tile import add_dep_helper
        add_dep_helper(insts["t0"].ins, insts["drain1"].ins, sync=True,
                       reason="slot the tile-1 PSUM drain before t0 on ACT")
        mono_phase(0)
        gram_phase(0, False, range(0, 32))
        gram_phase(0, True, range(32, 64))

        gout = mpool.tile([128, 128], f32, tag="gout")
        nc.vector.tensor_copy(gout[:, :], ps[:, :])
        nc.sync.dma_start(gram[:, :], gout[:, :])

    nc.compile()
    return nc


def _walsh16():
    sgn = 2.0 * ((np.arange(16)[:, None] >> np.arange(4)[None, :]) & 1) - 1.0
    w = np.ones((16, 16))
    for k in range(16):
        for s in range(16):
            v = 1.0
            for p in range(4):
                if s >> p & 1:
                    v *= sgn[k, p]
            w[k, s] = v
    return w


def _postprocess(grams):
    """grams: 8x [128,128] f32 -> [4,256,1,1]."""
    perm = np.argsort(PLANE_SUBSET)  # subset-index -> plane-index
    w16 = _walsh16()
    hi_sign = np.array([(-1.0) ** bin(s).count("1") for s in range(16)])
    out = np.zeros((4, 256), np.float64)
    for b in range(4):
        g16 = np.zeros((16, 16))
        for half in range(2):
            gr = grams[2 * b + half].astype(np.float64).reshape(16, 8, 16, 8)
            g = np.einsum("tgsg->st", gr)          # sum the 8 diagonal blocks
            g16 += g[np.ix_(perm, perm)]           # plane order -> subset order
        g16 *= hi_sign[None, :]                    # SA planes hold -z_{p+4}
        g16[0, 0] = float(HP * WP)                 # ones*ones: exact pixel count
        hmat = 2.0 ** -8 * (w16 @ g16 @ w16.T)     # [klo, khi]
        out[b] = hmat.T.reshape(256)               # k = klo + 16*khi
    return out.astype(np.float32).reshape(4, 256, 1, 1)


def _ident_np():
    import ml_dtypes
    a = np.zeros((128, 256), dtype=np.float32)
    for m in range(127):
        a[m + 1, m] = 1.0          # subdiagonal: out[m] = z[m+1]
    a[0, 128 + 127] = 1.0          # selector: out[127] = other-tile z[0]
    return a.astype(ml_dtypes.bfloat16)


def kernel(x, diff_hardness, output_hardness):
    global last_results
    from concourse.bass_utils import run_bass_kernel_spmd

    x = np.asarray(x, np.float32)
    dh = float(np.asarray(diff_hardness))
    oh = float(np.asarray(output_hardness))

    key = (dh, oh)
    if key not in _PROGRAM_CACHE:
        _PROGRAM_CACHE[key] = _build_program(dh, oh)
    nc = _PROGRAM_CACHE[key]

    ident = _ident_np()
    in_maps = []
    for core in range(8):
        b, half = divmod(core, 2)
        r0 = 0 if half == 0 else 255
        in_maps.append({
            "xs": np.ascontiguousarray(x[b, 0, r0:r0 + NROWS_SLICE, :]),
            "ident": ident,
        })

    trace = bool(int(os.environ.get("KERNEL_TRACE", "0")))
    res = run_bass_kernel_spmd(nc, in_maps, core_ids=list(range(8)), trace=trace)
    last_results = res
    grams = [res.results[c]["gram"] for c in range(8)]
    return _postprocess(grams)


# revision 23
# speedup vs baseline: 1.0201x; 1.0176x over previous
"""DiffLBP soft-histogram kernel for Trainium2 (8 NeuronCores).

Math: the per-pixel softmax over 256 LBP patterns factorizes exactly into a
product of 8 independent Bernoullis with q_p = 1/2 (1 + z_p),
z_p = tanh((oh/2)*tanh(dh*d_p)).  The histogram is a 16x16 Gram matrix of
z-monomials (4 low bits x 4 high bits) pushed through a constant Walsh +-1
transform (host).  Antipodal offsets give z_{p+4}(r,c) = -z_p((r,c)+off), so
only 4 z planes are computed; the "hi" side needs (row+1, col+dx) shifted
copies: both shifts are done by TensorE (subdiagonal-identity matmul with
col-offset APs into PSUM), drained to SBUF by one ScalarE copy per tile.

Device program per core (one batch b, one 255-row half; tile 1 first):
  z phase (x2 tiles of 128 rows): SWDGE DMA loads XA/XB row-windows cast to
    bf16; DVE computes the 4 diffs; ScalarE does two tanh passes, writing
    the z singles straight into the plane-major mlo tile (planes 1..4).
  shift (x2): TensorE multiplies z planes 0..2 by a subdiagonal identity
    with per-plane column offsets -> PSUM holds the row+col shifted hi
    singles (tile 0 also accumulates a selector matmul that injects
    z-tile-1 row 0 into row 127); ScalarE drains PSUM -> mhi planes 1..3.
  mono phase (x2): mlo is plane-major [128, 16, 512] (the gram's moving
    operand), mhi chunk-major [128, 64, 16x8] (the stationary needs a
    single-free-dim AP); the 11 composite monomial planes per side are
    built by 5 batched DVE multiplies.  The tile-1 PSUM drain is pinned
    between z1 and t0 on ScalarE via an explicit dep so the hi-side chain
    never waits behind the tile-0 tanh passes.
  gram phase (x2): 64 matmuls per tile accumulate into PSUM (tile-0's hi
    products and gram are split into chunk halves to shorten the tail).
    A zero-weight matmul warmup stream (reading z data so it cannot run
    too early) keeps the PE HAM un-throttled through the gram bursts.
    Tile 0 runs K=128 (the straddle row center 128 is included on-device:
    a selector matmul injects z-tile-1 row 0 into shift row 127), tile 1
    K=127 (its row 127 is the neighbouring core's center).  No host
    boundary fix needed.
"""

import os
import numpy as np
from contextlib import ExitStack

H = W = 512
HP = WP = 510          # valid center rows/cols
NROWS_SLICE = 257      # input rows per core slice

# plane slot -> subset bitmask of {z0,z1,z2,z3}; chosen so the 11 composite
# planes are produced by 5 batched multiplies (see _products)
PLANE_SUBSET = [0b0000,
                0b0001, 0b0010, 0b0100, 0b1000,   # 1..4:   z0 z1 z2 z3
                0b0011, 0b0110, 0b1100,           # 5..7:   {01} {12} {23}
                0b0101, 0b1010,                   # 8..9:   {02} {13}
                0b0111, 0b1110, 0b1111,           # 10..12: {012} {123} {0123}
                0b1011, 0b1101,                   # 13..14: {013} {023}
                0b1001]                           # 15:     {03}

_PROGRAM_CACHE = {}
last_results = None  # BassKernelResults of the most recent run (for test harness)


def _products(nc, m, R):
    """Emit the 11 composite monomial planes from singles (planes 1..4) of a
    plane-major view m[[part], 16, W]; writes planes 5..15 on R partitions.
    """
    nc.vector.tensor_mul(m[:R, 15:16], m[:R, 1:2], m[:R, 4:5])      # 03
    nc.vector.tensor_mul(m[:R, 5:8], m[:R, 1:4], m[:R, 2:5])        # 01 12 23
    nc.vector.tensor_mul(m[:R, 8:10], m[:R, 1:3], m[:R, 3:5])       # 02 13
    nc.vector.tensor_mul(m[:R, 10:13], m[:R, 5:8], m[:R, 3:6])      # 012 123 0123
    nc.vector.tensor_mul(m[:R, 13:15], m[:R, 9:7:-1], m[:R, 1:5:3]) # 013 023


def _build_program(dh: float, oh: float):
    import concourse.bacc as bacc
    import concourse.tile as tile
    from concourse import mybir
    import concourse.bass as bass

    f32 = mybir.dt.float32
    bf16 = mybir.dt.bfloat16
    Tanh = mybir.ActivationFunctionType.Tanh
    Copy = mybir.ActivationFunctionType.Copy

    nc = bacc.Bacc("TRN2", target_bir_lowering=False, debug=False)
    xs_t = nc.dram_tensor("xs", [NROWS_SLICE, W], f32, kind="ExternalInput")
    id_t = nc.dram_tensor("ident", [128, 256], bf16, kind="ExternalInput")
    gram = nc.dram_tensor("gram", [128, 128], f32, kind="ExternalOutput").ap()

    with tile.TileContext(nc) as tc, ExitStack() as ctx:
        xpool = ctx.enter_context(tc.tile_pool(name="x", bufs=2))
        dpool = ctx.enter_context(tc.tile_pool(name="d", bufs=2))
        tpool = ctx.enter_context(tc.tile_pool(name="t", bufs=1))
        mpool = ctx.enter_context(tc.tile_pool(name="m", bufs=1))
        ppool = ctx.enter_context(
            tc.tile_pool(name="ps", bufs=1, space=bass.MemorySpace.PSUM))

        # x loads first (they gate everything); SWDGE casts f32 -> bf16
        xts = {}
        for i in (1, 0):
            xt = xpool.tile([128, 2, W], bf16, name=f"xt{i}", tag=f"xt{i}")
            src = bass.AP(xs_t, 128 * i * W, [[W, 128], [W, 2], [1, W]])
            nc.gpsimd.dma_start(xt[:], src)
            xts[i] = xt

        # shifted-identity weights for the TensorE partition shift
        identt = mpool.tile([128, 256], bf16, tag="identt")
        nc.sync.dma_start(identt[:, :], id_t.ap())

        # trigger the tanh ACT table load immediately (overlaps the X DMAs)
        warm = mpool.tile([1, 8], f32, tag="warm")
        nc.vector.memset(warm[:, :], 0.0)
        nc.scalar.activation(warm[:, :], warm[:, :], Tanh)

        # zero stationary for the PE HAM warmup (contributes 0 to the gram)
        wz = mpool.tile([128, 128], bf16, tag="wz")
        nc.vector.memset(wz[:, :], 0.0)

        insts = {}
        ps = ppool.tile([128, 128], f32, tag="ps")
        pshift = {i: ppool.tile([128, 3, W], f32, name=f"pshift{i}",
                                tag=f"pshift{i}") for i in (0, 1)}
        mlos, mhis = {}, {}
        KS = {0: 128, 1: 127}   # gram contraction depth per tile

        # mlo: plane-major (the gram's MOVING operand tolerates 2 free
        # dims); z singles live in planes 1..4.  mhi: chunk-major [128, 64
        # chunks, 16 planes x 8 cols] (the STATIONARY needs 1 free dim).
        mhv = {}
        for i in (1, 0):
            mlos[i] = mpool.tile([128, 16, W], bf16, name=f"mlo{i}", tag=f"mlo{i}")
            mhis[i] = mpool.tile([128, 64, 128], bf16, name=f"mhi{i}", tag=f"mhi{i}")
            mhv[i] = mhis[i][:].rearrange("k g (s c) -> k s g c", c=8)
            nc.vector.memset(mlos[i][:, 0, :], 1.0)
            nc.vector.memset(mhis[i][:, :, 0:8], 1.0)

        def z_phase(i):
            xt = xts[i]
            xa, xb = xt[:, 0, :], xt[:, 1, :]
            d = dpool.tile([128, 4, W], bf16, name=f"d{i}", tag=f"d{i}")
            # zero the columns the subs below don't write (else NaN garbage
            # flows through tanh into the z edge cols)
            nc.vector.memset(d[:, 0, 0:1], 0.0)
            nc.vector.memset(d[:, 2:4, 511:512], 0.0)
            # d_p[cl] = X_{dy}[cl+dx] - XB[cl]   (cl = x-col = center_col + 1)
            nc.vector.tensor_sub(d[:, 0, 1:512], xa[:, 0:511], xb[:, 1:512])   # (-1,-1)
            nc.vector.tensor_sub(d[:, 1, 0:512], xa[:, 0:512], xb[:, 0:512])   # (-1, 0)
            nc.vector.tensor_sub(d[:, 2, 0:511], xa[:, 1:512], xb[:, 0:511])   # (-1,+1)
            nc.vector.tensor_sub(d[:, 3, 0:511], xb[:, 1:512], xb[:, 0:511])   # ( 0,+1)

            t = tpool.tile([128, 4, W], f32, name=f"t{i}", tag="t")
            nc.scalar.activation(t[:, 0:2, :], d[:, 0:2, :], Tanh,
                                 scale=float(dh))
            insts[f"t{i}"] = nc.scalar.activation(
                t[:, 2:4, :], d[:, 2:4, :], Tanh, scale=float(dh))
            # z singles straight into the plane-major mlo tile
            nc.scalar.activation(mlos[i][:, 1:5, :], t[:, :, :], Tanh,
                                 scale=float(oh) / 2.0)

        # col windows for the shift matmuls: plane p shifts cols by
        # dx'_p = (+1, 0, -1); out col c reads z col c+dx'
        SHIFT_WIN = [  # (out_lo, out_hi, in_lo, in_hi)
            (0, 511, 1, 512),
            (0, 512, 0, 512),
            (1, 512, 0, 511),
        ]

        def shift_phase(i):
            # pshift[i][rl, p, c] = z_p(row rl+1, col c+dx') via TensorE;
            # tile 0 row 127 = z-tile-1 row 0 via the selector weights.
            for p, (ol, ohi, il, ihi) in enumerate(SHIFT_WIN):
                nc.tensor.matmul(pshift[i][:, p, ol:ohi], identt[:, 0:128],
                                 mlos[i][:, 1 + p, il:ihi],
                                 start=True, stop=(i == 1))
            if i == 0:
                for p, (ol, ohi, il, ihi) in enumerate(SHIFT_WIN):
                    nc.tensor.matmul(pshift[0][:, p, ol:ohi], identt[:, 128:256],
                                     mlos[1][:, 1 + p, il:ihi],
                                     start=False, stop=True)

        def mono_phase(i):
            mlo, mh = mlos[i], mhv[i]
            R = KS[i]
            _products(nc, mlo[:], 128)
            # plane 4 of the hi side = z_3 col-shifted by -1, copied from mlo
            # plane 4 BEFORE the edge memset (the hi side needs the real
            # z3(cl=0); the copy's src col -1 lands on the cl=0 slot, which
            # the mhi edge memset overwrites)
            psr = pshift[i][0:R, :, :].rearrange("k p (g c) -> k p g c", c=8)
            p4src = (mlo[:, 3:5, :].rearrange("k p c -> k (p c)")[:, 511:1023]
                     .rearrange("k (g c) -> k g c", c=8))
            nc.vector.tensor_copy(mh[:, 4, :, :], p4src)
            # invalid edge columns: zero ALL lo planes at cl=0 and cl=511
            # (after products; kills the unwritten-d edge garbage too)
            nc.vector.memset(mlo[:, :, 0:512:511], 0.0)
            # hi singles 1..3: drain the shift PSUM (already row+col shifted)
            insts[f"drain{i}"] = nc.scalar.activation(mh[0:R, 1:4, :, :], psr, Copy)
            nc.vector.memset(mhis[i][:, 0, 8:40:8], 0.0)    # cl=0   singles
            nc.vector.memset(mhis[i][:, 63, 15:47:8], 0.0)  # cl=511 singles
            if i == 1:
                _products(nc, mh, R)
            else:
                # split tile 0's hi products by chunk half so the gram tail
                # can start after half A
                _products(nc, mh[:, :, 0:32, :], R)
                _products(nc, mh[:, :, 32:64, :], R)

        def warmup_phase(n, src_i, first=False):
            # zero-weight matmuls into the gram PSUM: keeps the PE HAM busy
            # so the gram bursts run at 2.4 GHz; contributes exactly 0.  The
            # moving operand reads z data so the warmup can't run too early.
            for k in range(n):
                nc.tensor.matmul(ps[:, :], wz[:, :], mlos[src_i][:, 2, 1:129],
                                 start=(first and k == 0), stop=False)

        def gram_phase(i, last, gs=range(64)):
            K = KS[i]
            for g in gs:
                nc.tensor.matmul(
                    ps[:, :],
                    mhis[i][0:K, g, :],
                    mlos[i][0:K, :, 8 * g:8 * g + 8],
                    start=False,
                    stop=(last and g == 63),
                )

        # tile 1 first: its shift is self-contained, so its gram unblocks
        # early; tile 0's shift needs z-tile-1 row 0 (the selector matmul)
        z_phase(1)
        z_phase(0)
        shift_phase(1)
        warmup_phase(50, 1, first=True)
        shift_phase(0)
        with tc.high_priority():
            mono_phase(1)
            gram_phase(1, False)
        from concourse.tile import add_dep_helper
        add_dep_helper(insts["t0"].ins, insts["drain1"].ins, sync=True,
                       reason="slot the tile-1 PSUM drain before t0 on ACT")
        mono_phase(0)
        gram_phase(0, False, range(0, 32))
        gram_phase(0, True, range(32, 64))

        gout = mpool.tile([128, 128], f32, tag="gout")
        nc.vector.tensor_copy(gout[:, :], ps[:, :])
        nc.sync.dma_start(gram[:, :], gout[:, :])

    nc.compile()
    return nc


def _walsh16():
    sgn = 2.0 * ((np.arange(16)[:, None] >> np.arange(4)[None, :]) & 1) - 1.0
    w = np.ones((16, 16))
    for k in range(16):
        for s in range(16):
            v = 1.0
            for p in range(4):
                if s >> p & 1:
                    v *= sgn[k, p]
            w[k, s] = v
    return w


def _postprocess(grams):
    """grams: 8x [128,128] f32 -> [4,256,1,1]."""
    perm = np.argsort(PLANE_SUBSET)  # subset-index -> plane-index
    w16 = _walsh16()
    hi_sign = np.array([(-1.0) ** bin(s).count("1") for s in range(16)])
    out = np.zeros((4, 256), np.float64)
    for b in range(4):
        g16 = np.zeros((16, 16))
        for half in range(2):
            gr = grams[2 * b + half].astype(np.float64).reshape(16, 8, 16, 8)
            g = np.einsum("tgsg->st", gr)          # sum the 8 diagonal blocks
            g16 += g[np.ix_(perm, perm)]           # plane order -> subset order
        g16 *= hi_sign[None, :]                    # SA planes hold -z_{p+4}
        g16[0, 0] = float(HP * WP)                 # ones*ones: exact pixel count
        hmat = 2.0 ** -8 * (w16 @ g16 @ w16.T)     # [klo, khi]
        out[b] = hmat.T.reshape(256)               # k = klo + 16*khi
    return out.astype(np.float32).reshape(4, 256, 1, 1)


def _ident_np():
    import ml_dtypes
    a = np.zeros((128, 256), dtype=np.float32)
    for m in range(127):
        a[m + 1, m] = 1.0          # subdiagonal: out[m] = z[m+1]
    a[0, 128 + 127] = 1.0          # selector: out[127] = other-tile z[0]
    return a.astype(ml_dtypes.bfloat16)


def kernel(x, diff_hardness, output_hardness):
    global last_results
    from concourse.bass_utils import run_bass_kernel_spmd

    x = np.asarray(x, np.float32)
    dh = float(np.asarray(diff_hardness))
    oh = float(np.asarray(output_hardness))

    key = (dh, oh)
    if key not in _PROGRAM_CACHE:
        _PROGRAM_CACHE[key] = _build_program(dh, oh)
    nc = _PROGRAM_CACHE[key]

    ident = _ident_np()
    in_maps = []
    for core in range(8):
        b, half = divmod(core, 2)
        r0 = 0 if half == 0 else 255
        in_maps.append({
            "xs": np.ascontiguousarray(x[b, 0, r0:r0 + NROWS_SLICE, :]),
            "ident": ident,
        })

    trace = bool(int(os.environ.get("KERNEL_TRACE", "0")))
    res = run_bass_kernel_spmd(nc, in_maps, core_ids=list(range(8)), trace=trace)
    last_results = res
    grams = [res.results[c]["gram"] for c in range(8)]
    return _postprocess(grams)


# revision 24
# speedup vs baseline: 1.0327x; 1.0124x over previous
"""DiffLBP soft-histogram kernel for Trainium2 (8 NeuronCores).

Math: the per-pixel softmax over 256 LBP patterns factorizes exactly into a
product of 8 independent Bernoullis with q_p = 1/2 (1 + z_p),
z_p = tanh((oh/2)*tanh(dh*d_p)).  The histogram is a 16x16 Gram matrix of
z-monomials (4 low bits x 4 high bits) pushed through a constant Walsh +-1
transform (host).  Antipodal offsets give z_{p+4}(r,c) = -z_p((r,c)+off), so
only 4 z planes are computed; the "hi" side needs (row+1, col+dx) shifted
copies: both shifts are done by TensorE (subdiagonal-identity matmul with
col-offset APs into PSUM), drained to SBUF by one ScalarE copy per tile.

Device program per core (one batch b, one 255-row half; tile 1 first):
  z phase (x2 tiles of 128 rows): SWDGE DMA loads XA/XB row-windows cast to
    bf16; DVE computes the 4 diffs; ScalarE does two tanh passes, writing
    the z singles straight into the plane-major mlo tile (planes 1..4).
  shift (x2): TensorE multiplies z planes 0..2 by a subdiagonal identity
    with per-plane column offsets -> PSUM holds the row+col shifted hi
    singles (tile 0 also accumulates a selector matmul that injects
    z-tile-1 row 0 into row 127); ScalarE drains PSUM -> mhi planes 1..3.
  mono phase (x2): mlo is plane-major [128, 16, 512] (the gram's moving
    operand), mhi chunk-major [128, 64, 16x8] (the stationary needs a
    single-free-dim AP); the 11 composite monomial planes per side are
    built by 5 batched DVE multiplies.  The tile-1 PSUM drain is pinned
    between z1 and t0 on ScalarE via an explicit dep so the hi-side chain
    never waits behind the tile-0 tanh passes.
  gram phase (x2): 64 matmuls per tile accumulate into PSUM (tile-0's hi
    products and gram are split into chunk halves to shorten the tail).
    A zero-weight matmul warmup stream (reading z data so it cannot run
    too early) keeps the PE HAM un-throttled through the gram bursts.
    Tile 0 runs K=128 (the straddle row center 128 is included on-device:
    a selector matmul injects z-tile-1 row 0 into shift row 127), tile 1
    K=127 (its row 127 is the neighbouring core's center).  No host
    boundary fix needed.
"""

import os
import numpy as np
from contextlib import ExitStack

H = W = 512
HP = WP = 510          # valid center rows/cols
NROWS_SLICE = 257      # input rows per core slice

# plane slot -> subset bitmask of {z0,z1,z2,z3}; chosen so the 11 composite
# planes are produced by 5 batched multiplies (see _products)
PLANE_SUBSET = [0b0000,
                0b0001, 0b0010, 0b0100, 0b1000,   # 1..4:   z0 z1 z2 z3
                0b0011, 0b0110, 0b1100,           # 5..7:   {01} {12} {23}
                0b0101, 0b1010,                   # 8..9:   {02} {13}
                0b0111, 0b1110, 0b1111,           # 10..12: {012} {123} {0123}
                0b1011, 0b1101,                   # 13..14: {013} {023}
                0b1001]                           # 15:     {03}

_PROGRAM_CACHE = {}
last_results = None  # BassKernelResults of the most recent run (for test harness)


def _products(nc, m, R):
    """Emit the 11 composite monomial planes from singles (planes 1..4) of a
    plane-major view m[[part], 16, W]; writes planes 5..15 on R partitions.
    """
    nc.vector.tensor_mul(m[:R, 15:16], m[:R, 1:2], m[:R, 4:5])      # 03
    nc.vector.tensor_mul(m[:R, 5:8], m[:R, 1:4], m[:R, 2:5])        # 01 12 23
    nc.vector.tensor_mul(m[:R, 8:10], m[:R, 1:3], m[:R, 3:5])       # 02 13
    nc.vector.tensor_mul(m[:R, 10:13], m[:R, 5:8], m[:R, 3:6])      # 012 123 0123
    nc.vector.tensor_mul(m[:R, 13:15], m[:R, 9:7:-1], m[:R, 1:5:3]) # 013 023


def _build_program(dh: float, oh: float):
    import concourse.bacc as bacc
    import concourse.tile as tile
    from concourse import mybir
    import concourse.bass as bass

    f32 = mybir.dt.float32
    bf16 = mybir.dt.bfloat16
    Tanh = mybir.ActivationFunctionType.Tanh
    Copy = mybir.ActivationFunctionType.Copy

    nc = bacc.Bacc("TRN2", target_bir_lowering=False, debug=False)
    xs_t = nc.dram_tensor("xs", [NROWS_SLICE, W], f32, kind="ExternalInput")
    id_t = nc.dram_tensor("ident", [128, 256], bf16, kind="ExternalInput")
    gram = nc.dram_tensor("gram", [128, 128], f32, kind="ExternalOutput").ap()

    with tile.TileContext(nc) as tc, ExitStack() as ctx:
        xpool = ctx.enter_context(tc.tile_pool(name="x", bufs=2))
        dpool = ctx.enter_context(tc.tile_pool(name="d", bufs=2))
        tpool = ctx.enter_context(tc.tile_pool(name="t", bufs=1))
        mpool = ctx.enter_context(tc.tile_pool(name="m", bufs=1))
        ppool = ctx.enter_context(
            tc.tile_pool(name="ps", bufs=1, space=bass.MemorySpace.PSUM))

        # x loads first (they gate everything); SWDGE casts f32 -> bf16
        xts = {}
        for i in (1, 0):
            xt = xpool.tile([128, 2, W], bf16, name=f"xt{i}", tag=f"xt{i}")
            src = bass.AP(xs_t, 128 * i * W, [[W, 128], [W, 2], [1, W]])
            nc.gpsimd.dma_start(xt[:], src)
            xts[i] = xt

        # shifted-identity weights for the TensorE partition shift
        identt = mpool.tile([128, 256], bf16, tag="identt")
        nc.sync.dma_start(identt[:, :], id_t.ap())

        # trigger the tanh ACT table load immediately (overlaps the X DMAs)
        warm = mpool.tile([1, 8], f32, tag="warm")
        nc.vector.memset(warm[:, :], 0.0)
        nc.scalar.activation(warm[:, :], warm[:, :], Tanh)

        # zero stationary for the PE HAM warmup (contributes 0 to the gram)
        wz = mpool.tile([128, 128], bf16, tag="wz")
        nc.vector.memset(wz[:, :], 0.0)

        insts = {}
        ps = ppool.tile([128, 128], f32, tag="ps")
        pshift = {i: ppool.tile([128, 3, W], f32, name=f"pshift{i}",
                                tag=f"pshift{i}") for i in (0, 1)}
        mlos, mhis = {}, {}
        KS = {0: 128, 1: 127}   # gram contraction depth per tile

        # mlo: plane-major (the gram's MOVING operand tolerates 2 free
        # dims); z singles live in planes 1..4.  mhi: chunk-major [128, 64
        # chunks, 16 planes x 8 cols] (the STATIONARY needs 1 free dim).
        mhv = {}
        for i in (1, 0):
            mlos[i] = mpool.tile([128, 16, W], bf16, name=f"mlo{i}", tag=f"mlo{i}")
            mhis[i] = mpool.tile([128, 64, 128], bf16, name=f"mhi{i}", tag=f"mhi{i}")
            mhv[i] = mhis[i][:].rearrange("k g (s c) -> k s g c", c=8)
            nc.vector.memset(mlos[i][:, 0, :], 1.0)
            nc.vector.memset(mhis[i][:, :, 0:8], 1.0)

        def z_phase(i):
            xt = xts[i]
            xa, xb = xt[:, 0, :], xt[:, 1, :]
            d = dpool.tile([128, 4, W], bf16, name=f"d{i}", tag=f"d{i}")
            # zero the columns the subs below don't write (else NaN garbage
            # flows through tanh into the z edge cols)
            nc.vector.memset(d[:, 0, 0:1], 0.0)
            nc.vector.memset(d[:, 2:4, 511:512], 0.0)
            # d_p[cl] = X_{dy}[cl+dx] - XB[cl]   (cl = x-col = center_col + 1)
            nc.vector.tensor_sub(d[:, 0, 1:512], xa[:, 0:511], xb[:, 1:512])   # (-1,-1)
            nc.vector.tensor_sub(d[:, 1, 0:512], xa[:, 0:512], xb[:, 0:512])   # (-1, 0)
            nc.vector.tensor_sub(d[:, 2, 0:511], xa[:, 1:512], xb[:, 0:511])   # (-1,+1)
            nc.vector.tensor_sub(d[:, 3, 0:511], xb[:, 1:512], xb[:, 0:511])   # ( 0,+1)

            t = tpool.tile([128, 4, W], f32, name=f"t{i}", tag="t")
            nc.scalar.activation(t[:, 0:2, :], d[:, 0:2, :], Tanh,
                                 scale=float(dh))
            insts[f"t{i}"] = nc.scalar.activation(
                t[:, 2:4, :], d[:, 2:4, :], Tanh, scale=float(dh))
            # z singles straight into the plane-major mlo tile; tile 0's
            # pass is split so the shift matmuls (planes 0..2) start early
            if i == 0:
                nc.scalar.activation(mlos[i][:, 1:4, :], t[:, 0:3, :], Tanh,
                                     scale=float(oh) / 2.0)
                nc.scalar.activation(mlos[i][:, 4:5, :], t[:, 3:4, :], Tanh,
                                     scale=float(oh) / 2.0)
            else:
                nc.scalar.activation(mlos[i][:, 1:5, :], t[:, :, :], Tanh,
                                     scale=float(oh) / 2.0)

        # col windows for the shift matmuls: plane p shifts cols by
        # dx'_p = (+1, 0, -1); out col c reads z col c+dx'
        SHIFT_WIN = [  # (out_lo, out_hi, in_lo, in_hi)
            (0, 511, 1, 512),
            (0, 512, 0, 512),
            (1, 512, 0, 511),
        ]

        def shift_phase(i):
            # pshift[i][rl, p, c] = z_p(row rl+1, col c+dx') via TensorE;
            # tile 0 row 127 = z-tile-1 row 0 via the selector weights.
            for p, (ol, ohi, il, ihi) in enumerate(SHIFT_WIN):
                nc.tensor.matmul(pshift[i][:, p, ol:ohi], identt[:, 0:128],
                                 mlos[i][:, 1 + p, il:ihi],
                                 start=True, stop=(i == 1))
            if i == 0:
                for p, (ol, ohi, il, ihi) in enumerate(SHIFT_WIN):
                    nc.tensor.matmul(pshift[0][:, p, ol:ohi], identt[:, 128:256],
                                     mlos[1][:, 1 + p, il:ihi],
                                     start=False, stop=True)

        def mono_phase(i):
            mlo, mh = mlos[i], mhv[i]
            R = KS[i]
            _products(nc, mlo[:], 128)
            # plane 4 of the hi side = z_3 col-shifted by -1, copied from mlo
            # plane 4 BEFORE the edge memset (the hi side needs the real
            # z3(cl=0); the copy's src col -1 lands on the cl=0 slot, which
            # the mhi edge memset overwrites)
            psr = pshift[i][0:R, :, :].rearrange("k p (g c) -> k p g c", c=8)
            p4src = (mlo[:, 3:5, :].rearrange("k p c -> k (p c)")[:, 511:1023]
                     .rearrange("k (g c) -> k g c", c=8))
            nc.vector.tensor_copy(mh[:, 4, :, :], p4src)
            # invalid edge columns: zero ALL lo planes at cl=0 and cl=511
            # (after products; kills the unwritten-d edge garbage too)
            nc.vector.memset(mlo[:, :, 0:512:511], 0.0)
            # hi singles 1..3: drain the shift PSUM (already row+col shifted)
            insts[f"drain{i}"] = nc.scalar.activation(mh[0:R, 1:4, :, :], psr, Copy)
            nc.vector.memset(mhis[i][:, 0, 8:40:8], 0.0)    # cl=0   singles
            nc.vector.memset(mhis[i][:, 63, 15:47:8], 0.0)  # cl=511 singles
            if i == 1:
                _products(nc, mh, R)
            else:
                # split tile 0's hi products by chunk half so the gram tail
                # can start after half A
                _products(nc, mh[:, :, 0:32, :], R)
                _products(nc, mh[:, :, 32:64, :], R)

        def warmup_phase(n, src_i, first=False):
            # zero-weight matmuls into the gram PSUM: keeps the PE HAM busy
            # so the gram bursts run at 2.4 GHz; contributes exactly 0.  The
            # moving operand reads z data so the warmup can't run too early.
            for k in range(n):
                nc.tensor.matmul(ps[:, :], wz[:, :], mlos[src_i][:, 2, 1:129],
                                 start=(first and k == 0), stop=False)

        def gram_phase(i, last, gs=range(64)):
            K = KS[i]
            for g in gs:
                nc.tensor.matmul(
                    ps[:, :],
                    mhis[i][0:K, g, :],
                    mlos[i][0:K, :, 8 * g:8 * g + 8],
                    start=False,
                    stop=(last and g == 63),
                )

        # tile 1 first: its shift is self-contained, so its gram unblocks
        # early; tile 0's shift needs z-tile-1 row 0 (the selector matmul)
        z_phase(1)
        z_phase(0)
        shift_phase(1)
        warmup_phase(50, 1, first=True)
        shift_phase(0)
        with tc.high_priority():
            mono_phase(1)
            gram_phase(1, False)
        from concourse.tile import add_dep_helper
        add_dep_helper(insts["t0"].ins, insts["drain1"].ins, sync=True,
                       reason="slot the tile-1 PSUM drain before t0 on ACT")
        mono_phase(0)
        gram_phase(0, False, range(0, 32))
        gram_phase(0, True, range(32, 64))

        gout = mpool.tile([128, 128], f32, tag="gout")
        nc.vector.tensor_copy(gout[:, :], ps[:, :])
        nc.sync.dma_start(gram[:, :], gout[:, :])

    nc.compile()
    return nc


def _walsh16():
    sgn = 2.0 * ((np.arange(16)[:, None] >> np.arange(4)[None, :]) & 1) - 1.0
    w = np.ones((16, 16))
    for k in range(16):
        for s in range(16):
            v = 1.0
            for p in range(4):
                if s >> p & 1:
                    v *= sgn[k, p]
            w[k, s] = v
    return w


def _postprocess(grams):
    """grams: 8x [128,128] f32 -> [4,256,1,1]."""
    perm = np.argsort(PLANE_SUBSET)  # subset-index -> plane-index
    w16 = _walsh16()
    hi_sign = np.array([(-1.0) ** bin(s).count("1") for s in range(16)])
    out = np.zeros((4, 256), np.float64)
    for b in range(4):
        g16 = np.zeros((16, 16))
        for half in range(2):
            gr = grams[2 * b + half].astype(np.float64).reshape(16, 8, 16, 8)
            g = np.einsum("tgsg->st", gr)          # sum the 8 diagonal blocks
            g16 += g[np.ix_(perm, perm)]           # plane order -> subset order
        g16 *= hi_sign[None, :]                    # SA planes hold -z_{p+4}
        g16[0, 0] = float(HP * WP)                 # ones*ones: exact pixel count
        hmat = 2.0 ** -8 * (w16 @ g16 @ w16.T)     # [klo, khi]
        out[b] = hmat.T.reshape(256)               # k = klo + 16*khi
    return out.astype(np.float32).reshape(4, 256, 1, 1)


def _ident_np():
    import ml_dtypes
    a = np.zeros((128, 256), dtype=np.float32)
    for m in range(127):
        a[m + 1, m] = 1.0          # subdiagonal: out[m] = z[m+1]
    a[0, 128 + 127] = 1.0          # selector: out[127] = other-tile z[0]
    return a.astype(ml_dtypes.bfloat16)


def kernel(x, diff_hardness, output_hardness):
    global last_results
    from concourse.bass_utils import run_bass_kernel_spmd

    x = np.asarray(x, np.float32)
    dh = float(np.asarray(diff_hardness))
    oh = float(np.asarray(output_hardness))

    key = (dh, oh)
    if key not in _PROGRAM_CACHE:
        _PROGRAM_CACHE[key] = _build_program(dh, oh)
    nc = _PROGRAM_CACHE[key]

    ident = _ident_np()
    in_maps = []
    for core in range(8):
        b, half = divmod(core, 2)
        r0 = 0 if half == 0 else 255
        in_maps.append({
            "xs": np.ascontiguousarray(x[b, 0, r0:r0 + NROWS_SLICE, :]),
            "ident": ident,
        })

    trace = bool(int(os.environ.get("KERNEL_TRACE", "0")))
    res = run_bass_kernel_spmd(nc, in_maps, core_ids=list(range(8)), trace=trace)
    last_results = res
    grams = [res.results[c]["gram"] for c in range(8)]
    return _postprocess(grams)


# revision 25
# speedup vs baseline: 1.0345x; 1.0018x over previous
"""DiffLBP soft-histogram kernel for Trainium2 (8 NeuronCores).

Math: the per-pixel softmax over 256 LBP patterns factorizes exactly into a
product of 8 independent Bernoullis with q_p = 1/2 (1 + z_p),
z_p = tanh((oh/2)*tanh(dh*d_p)).  The histogram is a 16x16 Gram matrix of
z-monomials (4 low bits x 4 high bits) pushed through a constant Walsh +-1
transform (host).  Antipodal offsets give z_{p+4}(r,c) = -z_p((r,c)+off), so
only 4 z planes are computed; the "hi" side needs (row+1, col+dx) shifted
copies: both shifts are done by TensorE (subdiagonal-identity matmul with
col-offset APs into PSUM), drained to SBUF by one ScalarE copy per tile.

Device program per core (one batch b, one 255-row half; tile 1 first):
  z phase (x2 tiles of 128 rows): SWDGE DMA loads XA/XB row-windows cast to
    bf16; DVE computes the 4 diffs; ScalarE does two tanh passes, writing
    the z singles straight into the plane-major mlo tile (planes 1..4).
  shift (x2): TensorE multiplies z planes 0..2 by a subdiagonal identity
    with per-plane column offsets -> PSUM holds the row+col shifted hi
    singles (tile 0 also accumulates a selector matmul that injects
    z-tile-1 row 0 into row 127); ScalarE drains PSUM -> mhi planes 1..3.
  mono phase (x2): mlo is plane-major [128, 16, 512] (the gram's moving
    operand), mhi chunk-major [128, 64, 16x8] (the stationary needs a
    single-free-dim AP); the 11 composite monomial planes per side are
    built by 5 batched DVE multiplies.  The tile-1 PSUM drain is pinned
    between z1 and t0 on ScalarE via an explicit dep so the hi-side chain
    never waits behind the tile-0 tanh passes.
  gram phase (x2): 64 matmuls per tile accumulate into PSUM (tile-0's hi
    products and gram are split into chunk halves to shorten the tail).
    A zero-weight matmul warmup stream (reading z data so it cannot run
    too early) keeps the PE HAM un-throttled through the gram bursts.
    Tile 0 runs K=128 (the straddle row center 128 is included on-device:
    a selector matmul injects z-tile-1 row 0 into shift row 127), tile 1
    K=127 (its row 127 is the neighbouring core's center).  No host
    boundary fix needed.
"""

import os
import numpy as np
from contextlib import ExitStack

H = W = 512
HP = WP = 510          # valid center rows/cols
NROWS_SLICE = 257      # input rows per core slice

# plane slot -> subset bitmask of {z0,z1,z2,z3}; chosen so the 11 composite
# planes are produced by 5 batched multiplies (see _products)
PLANE_SUBSET = [0b0000,
                0b0001, 0b0010, 0b0100, 0b1000,   # 1..4:   z0 z1 z2 z3
                0b0011, 0b0110, 0b1100,           # 5..7:   {01} {12} {23}
                0b0101, 0b1010,                   # 8..9:   {02} {13}
                0b0111, 0b1110, 0b1111,           # 10..12: {012} {123} {0123}
                0b1011, 0b1101,                   # 13..14: {013} {023}
                0b1001]                           # 15:     {03}

_PROGRAM_CACHE = {}
last_results = None  # BassKernelResults of the most recent run (for test harness)


def _products(nc, m, R):
    """Emit the 11 composite monomial planes from singles (planes 1..4) of a
    plane-major view m[[part], 16, W]; writes planes 5..15 on R partitions.
    """
    nc.vector.tensor_mul(m[:R, 15:16], m[:R, 1:2], m[:R, 4:5])      # 03
    nc.vector.tensor_mul(m[:R, 5:8], m[:R, 1:4], m[:R, 2:5])        # 01 12 23
    nc.vector.tensor_mul(m[:R, 8:10], m[:R, 1:3], m[:R, 3:5])       # 02 13
    nc.vector.tensor_mul(m[:R, 10:13], m[:R, 5:8], m[:R, 3:6])      # 012 123 0123
    nc.vector.tensor_mul(m[:R, 13:15], m[:R, 9:7:-1], m[:R, 1:5:3]) # 013 023


def _build_program(dh: float, oh: float):
    import concourse.bacc as bacc
    import concourse.tile as tile
    from concourse import mybir
    import concourse.bass as bass

    f32 = mybir.dt.float32
    bf16 = mybir.dt.bfloat16
    Tanh = mybir.ActivationFunctionType.Tanh
    Copy = mybir.ActivationFunctionType.Copy

    nc = bacc.Bacc("TRN2", target_bir_lowering=False, debug=False)
    xs_t = nc.dram_tensor("xs", [NROWS_SLICE, W], f32, kind="ExternalInput")
    id_t = nc.dram_tensor("ident", [128, 256], bf16, kind="ExternalInput")
    gram = nc.dram_tensor("gram", [128, 128], f32, kind="ExternalOutput").ap()

    with tile.TileContext(nc) as tc, ExitStack() as ctx:
        xpool = ctx.enter_context(tc.tile_pool(name="x", bufs=2))
        dpool = ctx.enter_context(tc.tile_pool(name="d", bufs=2))
        tpool = ctx.enter_context(tc.tile_pool(name="t", bufs=1))
        mpool = ctx.enter_context(tc.tile_pool(name="m", bufs=1))
        ppool = ctx.enter_context(
            tc.tile_pool(name="ps", bufs=1, space=bass.MemorySpace.PSUM))

        # x loads first (they gate everything); SWDGE casts f32 -> bf16
        xts = {}
        for i in (1, 0):
            xt = xpool.tile([128, 2, W], bf16, name=f"xt{i}", tag=f"xt{i}")
            src = bass.AP(xs_t, 128 * i * W, [[W, 128], [W, 2], [1, W]])
            nc.gpsimd.dma_start(xt[:], src)
            xts[i] = xt

        # shifted-identity weights for the TensorE partition shift
        identt = mpool.tile([128, 256], bf16, tag="identt")
        nc.sync.dma_start(identt[:, :], id_t.ap())

        # trigger the tanh ACT table load immediately (overlaps the X DMAs)
        warm = mpool.tile([1, 8], f32, tag="warm")
        nc.vector.memset(warm[:, :], 0.0)
        nc.scalar.activation(warm[:, :], warm[:, :], Tanh)

        # zero stationary for the PE HAM warmup (contributes 0 to the gram)
        wz = mpool.tile([128, 128], bf16, tag="wz")
        nc.vector.memset(wz[:, :], 0.0)

        insts = {}
        ps = ppool.tile([128, 128], f32, tag="ps")
        pshift = {i: ppool.tile([128, 3, W], f32, name=f"pshift{i}",
                                tag=f"pshift{i}") for i in (0, 1)}
        mlos, mhis = {}, {}
        KS = {0: 128, 1: 127}   # gram contraction depth per tile

        # mlo: plane-major (the gram's MOVING operand tolerates 2 free
        # dims); z singles live in planes 1..4.  mhi: chunk-major [128, 64
        # chunks, 16 planes x 8 cols] (the STATIONARY needs 1 free dim).
        mhv = {}
        for i in (1, 0):
            mlos[i] = mpool.tile([128, 16, W], bf16, name=f"mlo{i}", tag=f"mlo{i}")
            mhis[i] = mpool.tile([128, 64, 128], bf16, name=f"mhi{i}", tag=f"mhi{i}")
            mhv[i] = mhis[i][:].rearrange("k g (s c) -> k s g c", c=8)
            nc.vector.memset(mlos[i][:, 0, :], 1.0)
            nc.vector.memset(mhis[i][:, :, 0:8], 1.0)

        def z_phase(i):
            xt = xts[i]
            xa, xb = xt[:, 0, :], xt[:, 1, :]
            d = dpool.tile([128, 4, W], bf16, name=f"d{i}", tag=f"d{i}")
            # zero the columns the subs below don't write (else NaN garbage
            # flows through tanh into the z edge cols)
            nc.vector.memset(d[:, 0, 0:1], 0.0)
            nc.vector.memset(d[:, 2:4, 511:512], 0.0)
            # d_p[cl] = X_{dy}[cl+dx] - XB[cl]   (cl = x-col = center_col + 1)
            nc.vector.tensor_sub(d[:, 0, 1:512], xa[:, 0:511], xb[:, 1:512])   # (-1,-1)
            nc.vector.tensor_sub(d[:, 1, 0:512], xa[:, 0:512], xb[:, 0:512])   # (-1, 0)
            nc.vector.tensor_sub(d[:, 2, 0:511], xa[:, 1:512], xb[:, 0:511])   # (-1,+1)
            nc.vector.tensor_sub(d[:, 3, 0:511], xb[:, 1:512], xb[:, 0:511])   # ( 0,+1)

            t = tpool.tile([128, 4, W], f32, name=f"t{i}", tag="t")
            nc.scalar.activation(t[:, 0:2, :], d[:, 0:2, :], Tanh,
                                 scale=float(dh))
            insts[f"t{i}"] = nc.scalar.activation(
                t[:, 2:4, :], d[:, 2:4, :], Tanh, scale=float(dh))
            # z singles straight into the plane-major mlo tile; tile 0's
            # pass is split so the shift matmuls (planes 0..2) start early
            if i == 0:
                nc.scalar.activation(mlos[i][:, 1:4, :], t[:, 0:3, :], Tanh,
                                     scale=float(oh) / 2.0)
                nc.scalar.activation(mlos[i][:, 4:5, :], t[:, 3:4, :], Tanh,
                                     scale=float(oh) / 2.0)
            else:
                nc.scalar.activation(mlos[i][:, 1:5, :], t[:, :, :], Tanh,
                                     scale=float(oh) / 2.0)

        # col windows for the shift matmuls: plane p shifts cols by
        # dx'_p = (+1, 0, -1); out col c reads z col c+dx'
        SHIFT_WIN = [  # (out_lo, out_hi, in_lo, in_hi)
            (0, 511, 1, 512),
            (0, 512, 0, 512),
            (1, 512, 0, 511),
        ]

        def shift_phase(i):
            # pshift[i][rl, p, c] = z_p(row rl+1, col c+dx') via TensorE;
            # tile 0 row 127 = z-tile-1 row 0 via the selector weights.
            for p, (ol, ohi, il, ihi) in enumerate(SHIFT_WIN):
                nc.tensor.matmul(pshift[i][:, p, ol:ohi], identt[:, 0:128],
                                 mlos[i][:, 1 + p, il:ihi],
                                 start=True, stop=(i == 1))
            if i == 0:
                for p, (ol, ohi, il, ihi) in enumerate(SHIFT_WIN):
                    nc.tensor.matmul(pshift[0][:, p, ol:ohi], identt[:, 128:256],
                                     mlos[1][:, 1 + p, il:ihi],
                                     start=False, stop=True)

        def mono_phase(i):
            mlo, mh = mlos[i], mhv[i]
            R = KS[i]
            _products(nc, mlo[:], 128)
            # plane 4 of the hi side = z_3 col-shifted by -1, copied from mlo
            # plane 4 BEFORE the edge memset (the hi side needs the real
            # z3(cl=0); the copy's src col -1 lands on the cl=0 slot, which
            # the mhi edge memset overwrites)
            psr = pshift[i][0:R, :, :].rearrange("k p (g c) -> k p g c", c=8)
            p4src = (mlo[:, 3:5, :].rearrange("k p c -> k (p c)")[:, 511:1023]
                     .rearrange("k (g c) -> k g c", c=8))
            nc.vector.tensor_copy(mh[:, 4, :, :], p4src)
            # invalid edge columns: zero ALL lo planes at cl=0 and cl=511
            # (after products; kills the unwritten-d edge garbage too)
            nc.vector.memset(mlo[:, :, 0:512:511], 0.0)
            # hi singles 1..3: drain the shift PSUM (already row+col shifted)
            insts[f"drain{i}"] = nc.scalar.activation(mh[0:R, 1:4, :, :], psr, Copy)
            nc.vector.memset(mhis[i][:, 0, 8:40:8], 0.0)    # cl=0   singles
            nc.vector.memset(mhis[i][:, 63, 15:47:8], 0.0)  # cl=511 singles
            if i == 1:
                _products(nc, mh, R)
            else:
                # split tile 0's hi products by chunk half so the gram tail
                # can start after half A
                _products(nc, mh[:, :, 0:32, :], R)
                _products(nc, mh[:, :, 32:64, :], R)

        def warmup_phase(n, src_i, first=False):
            # zero-weight matmuls into the gram PSUM: keeps the PE HAM busy
            # so the gram bursts run at 2.4 GHz; contributes exactly 0.  The
            # moving operand reads z data so the warmup can't run too early.
            for k in range(n):
                nc.tensor.matmul(ps[:, :], wz[:, :], mlos[src_i][:, 2, 1:129],
                                 start=(first and k == 0), stop=False)

        def gram_phase(i, last, gs=range(64)):
            K = KS[i]
            for g in gs:
                nc.tensor.matmul(
                    ps[:, :],
                    mhis[i][0:K, g, :],
                    mlos[i][0:K, :, 8 * g:8 * g + 8],
                    start=False,
                    stop=(last and g == 63),
                )

        # tile 1 first: its shift is self-contained, so its gram unblocks
        # early; tile 0's shift needs z-tile-1 row 0 (the selector matmul)
        z_phase(1)
        z_phase(0)
        shift_phase(1)
        warmup_phase(38, 1, first=True)
        shift_phase(0)
        with tc.high_priority():
            mono_phase(1)
            gram_phase(1, False)
        from concourse.tile import add_dep_helper
        add_dep_helper(insts["t0"].ins, insts["drain1"].ins, sync=True,
                       reason="slot the tile-1 PSUM drain before t0 on ACT")
        mono_phase(0)
        gram_phase(0, False, range(0, 32))
        gram_phase(0, True, range(32, 64))

        gout = mpool.tile([128, 128], f32, tag="gout")
        nc.vector.tensor_copy(gout[:, :], ps[:, :])
        nc.sync.dma_start(gram[:, :], gout[:, :])

    nc.compile()
    return nc


def _walsh16():
    sgn = 2.0 * ((np.arange(16)[:, None] >> np.arange(4)[None, :]) & 1) - 1.0
    w = np.ones((16, 16))
    for k in range(16):
        for s in range(16):
            v = 1.0
            for p in range(4):
                if s >> p & 1:
                    v *= sgn[k, p]
            w[k, s] = v
    return w


def _postprocess(grams):
    """grams: 8x [128,128] f32 -> [4,256,1,1]."""
    perm = np.argsort(PLANE_SUBSET)  # subset-index -> plane-index
    w16 = _walsh16()
    hi_sign = np.array([(-1.0) ** bin(s).count("1") for s in range(16)])
    out = np.zeros((4, 256), np.float64)
    for b in range(4):
        g16 = np.zeros((16, 16))
        for half in range(2):
            gr = grams[2 * b + half].astype(np.float64).reshape(16, 8, 16, 8)
            g = np.einsum("tgsg->st", gr)          # sum the 8 diagonal blocks
            g16 += g[np.ix_(perm, perm)]           # plane order -> subset order
        g16 *= hi_sign[None, :]                    # SA planes hold -z_{p+4}
        g16[0, 0] = float(HP * WP)                 # ones*ones: exact pixel count
        hmat = 2.0 ** -8 * (w16 @ g16 @ w16.T)     # [klo, khi]
        out[b] = hmat.T.reshape(256)               # k = klo + 16*khi
    return out.astype(np.float32).reshape(4, 256, 1, 1)


def _ident_np():
    import ml_dtypes
    a = np.zeros((128, 256), dtype=np.float32)
    for m in range(127):
        a[m + 1, m] = 1.0          # subdiagonal: out[m] = z[m+1]
    a[0, 128 + 127] = 1.0          # selector: out[127] = other-tile z[0]
    return a.astype(ml_dtypes.bfloat16)


def kernel(x, diff_hardness, output_hardness):
    global last_results
    from concourse.bass_utils import run_bass_kernel_spmd

    x = np.asarray(x, np.float32)
    dh = float(np.asarray(diff_hardness))
    oh = float(np.asarray(output_hardness))

    key = (dh, oh)
    if key not in _PROGRAM_CACHE:
        _PROGRAM_CACHE[key] = _build_program(dh, oh)
    nc = _PROGRAM_CACHE[key]

    ident = _ident_np()
    in_maps = []
    for core in range(8):
        b, half = divmod(core, 2)
        r0 = 0 if half == 0 else 255
        in_maps.append({
            "xs": np.ascontiguousarray(x[b, 0, r0:r0 + NROWS_SLICE, :]),
            "ident": ident,
        })

    trace = bool(int(os.environ.get("KERNEL_TRACE", "0")))
    res = run_bass_kernel_spmd(nc, in_maps, core_ids=list(range(8)), trace=trace)
    last_results = res
    grams = [res.results[c]["gram"] for c in range(8)]
    return _postprocess(grams)
